# revision 5
# baseline (speedup 1.0000x reference)
"""Trainium2 Bass kernel for y = x @ W^T + b  (B=4096, IN=OUT=2048, fp32).

Sharding: 4-way split on batch x 2-way split on out_features across the 8
NeuronCores.  Each core computes a [1024, 1024] block of the output from
x^T shard [2048, 1024] and W^T shard [2048, 1024] (both pre-transposed and
rounded to bf16 on the host; K lands on SBUF partitions with contiguous
DMAs).  bf16 inputs stream the PE at the same 1 row/cycle as fp32r but
halve HBM traffic; PSUM accumulates in fp32 so the K=2048 reduction loses
nothing, and the bf16 output rounding keeps rel-err ~4e-3.

Structure (per core, 8 PSUM banks = 4 m-tiles x 2 n-tiles per phase):
 - phase A (m-tiles 0-3): k-OUTER loop so each k-tile's w/x DMA only has
   to land just before its 8 matmuls -- compute starts after ~160KB.
 - phase B (m-tiles 4-7): k-INNER per bank (all inputs are resident by
   then), so banks complete progressively and the bias-add + store drain
   overlaps the remaining matmuls instead of serializing at the end.
 - DMA: w on the SP HWDGE queue, x/bias on the Activation HWDGE queue,
   phase-A stores on the gpsimd SWDGE queue, phase-B stores back on the
   Activation queue (idle by then).

Constraint driving the sync passes: a Matmult instruction on TRN2
supports only ONE sync-wait; extra waits are split into EventSemaphore
prefixes on the PE queue, and redundant PE self-waits are dropped.
"""

import os

import numpy as np

P = 128
B, IN, OUT = 4096, 2048, 2048
MB_SPLIT, NB_SPLIT = 4, 2  # batch-split x out-split = 8 cores
BM = B // MB_SPLIT  # 1024 batch rows per core
NO = OUT // NB_SPLIT  # 1024 out cols per core
KT = IN // P  # 16 k-tiles
MT = BM // P  # 8 m-tiles
NFREE = 512  # PSUM bank free dim (fp32)
NT = NO // NFREE  # 2 n-tiles
N_CORES = 8
HALF = (MT // 2) * P  # 512 x^T cols per phase

MM_DT = os.environ.get("BASS_MM_DT", "bfloat16")
OUT_DT = os.environ.get("BASS_OUT_DT", "bfloat16")

_CACHE = {}


def _build(mm_dt_name: str):
    import concourse.bass as bass
    import concourse.mybir as mybir
    import concourse.tile as tile

    mmdt = getattr(mybir.dt, mm_dt_name)
    outdt = getattr(mybir.dt, OUT_DT)
    f32 = mybir.dt.float32

    nc = bass.Bass("TRN2", target_bir_lowering=False, debug=False,
                   num_devices=N_CORES)
    xt = nc.dram_tensor("xt", [IN, BM], mmdt, kind="ExternalInput")
    wt = nc.dram_tensor("wt", [IN, NO], mmdt, kind="ExternalInput")
    bi = nc.dram_tensor("bi", [NO], f32, kind="ExternalInput")
    y = nc.dram_tensor("y", [BM, NO], outdt, kind="ExternalOutput")

    xt_r = xt.ap().rearrange("(k p) m -> p k m", p=P)  # [128, 16, 1024]
    wt_r = wt.ap().rearrange("(k p) n -> p k n", p=P)
    y_ap = y.ap()

    # group order within a phase: n fastest, so the DVE drain order
    # (m-major) matches phase B's bank-completion order
    groups = [(m, n) for m in range(MT // 2) for n in range(NT)]

    with tile.TileContext(nc) as tc:
        with (
            tc.tile_pool(name="xp", bufs=1) as xp,
            tc.tile_pool(name="wp", bufs=1) as wp,
            tc.tile_pool(name="bp", bufs=1) as bp,
            tc.tile_pool(name="op", bufs=1) as op,
            tc.tile_pool(name="ps", bufs=1, space="PSUM") as ps,
        ):
            # ---- input DMA emission ----
            # w stream on the SP (sync) HWDGE queue; x + bias on the
            # Activation (scalar) HWDGE queue.  k0 pieces are split small
            # so the first matmuls wait on ~160KB, not 768KB.
            wk = [None] * KT  # phase rhs tiles; wk[0] stays None (split)
            w0n = [None] * NT
            xak = [None] * KT  # xak[0] stays None (split into m0 + rest)

            for n in range(NT):
                t = wp.tile([P, NFREE], mmdt, tag=f"w0n{n}", name=f"w0n{n}")
                nc.sync.dma_start(t[:], wt_r[:, 0, n * NFREE:(n + 1) * NFREE])
                w0n[n] = t
            for k in range(2, KT):
                t = wp.tile([P, NO], mmdt, tag=f"wk{k}", name=f"wk{k}")
                nc.sync.dma_start(t[:], wt_r[:, k, :])
                wk[k] = t

            xa0m0 = xp.tile([P, P], mmdt, tag="xa0m0", name="xa0m0")
            nc.scalar.dma_start(xa0m0[:], xt_r[:, 0, :P])
            xa0r = xp.tile([P, HALF - P], mmdt, tag="xa0r", name="xa0r")
            nc.scalar.dma_start(xa0r[:], xt_r[:, 0, P:HALF])
            for k in range(2, KT):
                t = xp.tile([P, HALF], mmdt, tag=f"xak{k}", name=f"xak{k}")
                nc.scalar.dma_start(t[:], xt_r[:, k, :HALF])
                xak[k] = t
            bias_sb = bp.tile([P, NO], f32, tag="bias")
            nc.scalar.dma_start(bias_sb[:],
                                bi.ap()[None, :].to_broadcast((P, NO)))
            # all of phase B's x in one DMA (resident well before needed)
            xb_all = xp.tile([P, KT * HALF], mmdt, tag="xb", name="xb_all")
            nc.scalar.dma_start(
                xb_all[:].rearrange("p (k m) -> p k m", k=KT),
                xt_r[:, :, HALF:])

            # k1's tiles on the gpsimd SWDGE queue (idle until the phase-A
            # drain) so the first k-blocks never starve the PE
            t = wp.tile([P, NO], mmdt, tag="wk1", name="wk1")
            nc.gpsimd.dma_start(t[:], wt_r[:, 1, :])
            wk[1] = t
            t = xp.tile([P, HALF], mmdt, tag="xak1", name="xak1")
            nc.gpsimd.dma_start(t[:], xt_r[:, 1, :HALF])
            xak[1] = t

            def lhs_a(k, m):
                if k == 0:
                    return xa0m0[:] if m == 0 else \
                        xa0r[:, (m - 1) * P:m * P]
                return xak[k][:, m * P:(m + 1) * P]

            def rhs_w(k, n):
                if k == 0:
                    return w0n[n][:]
                return wk[k][:, n * NFREE:(n + 1) * NFREE]

            psum = {}
            for gi, g in enumerate(groups):
                psum[g] = ps.tile([P, NFREE], f32, tag=f"ps{gi}",
                                  name=f"psum_a_{gi}")

            # ---- phase A: k-outer ----
            # k0 runs all n0 groups first: w0n1 arrives one transfer after
            # w0n0, so the n0 block covers the gap.
            groups_k0 = [(m, n) for n in range(NT) for m in range(MT // 2)]
            for k in range(KT):
                for m, n in (groups_k0 if k == 0 else groups):
                    nc.tensor.matmul(
                        psum[(m, n)][:],
                        lhsT=lhs_a(k, m),
                        rhs=rhs_w(k, n),
                        start=(k == 0),
                        stop=(k == KT - 1),
                    )
            # phase A drain: one [128, NO] out tile per m; emitted in the
            # same (m, n) order as phase B consumes the banks.
            for m in range(MT // 2):
                ot = op.tile([P, NO], outdt, tag=f"outA{m}",
                             name=f"out_a_{m}")
                for n in range(NT):
                    nc.vector.tensor_add(
                        ot[:, n * NFREE:(n + 1) * NFREE],
                        psum[(m, n)][:],
                        bias_sb[:, n * NFREE:(n + 1) * NFREE])
                nc.gpsimd.dma_start(y_ap[m * P:(m + 1) * P, :], ot[:])

            # ---- phase B: k-inner per bank, progressive drain ----
            # per-(m, n) out tiles + stores so each bank's result ships as
            # soon as its add finishes; the final store is only 128KB.
            for gi, (m, n) in enumerate(groups):
                t = ps.tile([P, NFREE], f32, tag=f"ps{gi}",
                            name=f"psum_b_{gi}")
                for k in range(KT):
                    nc.tensor.matmul(
                        t[:],
                        lhsT=xb_all[:, k * HALF + m * P:
                                    k * HALF + (m + 1) * P],
                        rhs=rhs_w(k, n),
                        start=(k == 0),
                        stop=(k == KT - 1),
                    )
                ot = op.tile([P, NFREE], outdt, tag=f"outB{gi}",
                             name=f"out_b_{gi}")
                nc.vector.tensor_add(
                    ot[:], t[:],
                    bias_sb[:, n * NFREE:(n + 1) * NFREE])
                row0 = (MT // 2 + m) * P
                nc.scalar.dma_start(
                    y_ap[row0:row0 + P, n * NFREE:(n + 1) * NFREE], ot[:])

    _strip_redundant_pe_waits(nc)
    _legalize_multi_waits(nc)
    _check_matmul_waits(nc)
    return nc


def _legalize_multi_waits(nc):
    """Split multi-wait instructions into single-wait EventSemaphore
    prefixes on the same engine.

    This walrus pipeline (bass pass list, no lower_sync) supports exactly
    ONE sync wait per instruction.  A chain of EventSemaphore waits on the
    issuing engine followed by the instruction with the final wait is
    semantically identical: the engine's sequencer blocks on each in
    order.
    """
    import copy

    import concourse.mybir as mybir

    m = nc.m
    new_module = copy.replace(m, functions=[])
    counter = [0]
    for function in m.functions:
        new_function = copy.replace(function, blocks=[])
        new_function.set_allocations_from_list(function.allocations)
        for block in function.blocks:
            new_insts = []
            for inst in block.instructions:
                s = inst.sync_info
                if s and s.on_wait and len(s.on_wait) > 1:
                    for w in s.on_wait[:-1]:
                        counter[0] += 1
                        ev = mybir.InstEventSemaphore(
                            name=f"legalize_wait_{counter[0]}",
                            ins=[], outs=[],
                            sync_info=mybir.SyncInfo(on_wait=[w],
                                                     on_update=[]),
                            engine=inst.engine,
                        )
                        new_insts.append(ev)
                    inst.sync_info = mybir.SyncInfo(
                        on_wait=[s.on_wait[-1]], on_update=s.on_update)
                new_insts.append(inst)
            new_function.blocks.append(
                copy.replace(block, instructions=new_insts))
        new_module.functions.append(new_function)
    nc.m = new_module


def _strip_redundant_pe_waits(nc):
    """Drop PE self-waits on matmuls that also wait on the DVE release.

    TRN2 matmuls support one sync wait.  Tile's wait emission is not
    transitively minimal: a PSUM-bank reuse emits both the bank's last PE
    writer (self-engine, redundant: the DVE add that releases the bank
    already waits on that writer) and the DVE release.  Keeping the DVE
    wait preserves the hazard ordering.
    """
    import concourse.mybir as mybir

    for bb in nc.m.functions[0].blocks:
        for inst in bb.instructions:
            if type(inst).__name__ != "InstMatmult":
                continue
            s = inst.sync_info
            if not (s and s.on_wait and len(s.on_wait) > 1):
                continue
            keep = [w for w in s.on_wait if not w.ant_name.startswith("PE")]
            dve = [w for w in keep if w.ant_name.startswith("DVE")]
            if len(keep) == len(s.on_wait) - 1 and dve:
                inst.sync_info = mybir.SyncInfo(on_wait=keep,
                                                on_update=s.on_update)


def _check_matmul_waits(nc):
    """TRN2 compute instructions (Matmult, TensorTensor, ...) support one
    sync wait; walrus codegen hard-fails on more."""
    limited = {"InstMatmult", "InstTensorTensor", "InstTensorScalarPtr",
               "InstActivation", "InstTensorCopy", "InstCopy"}
    bad = []
    for bb in nc.m.functions[0].blocks:
        for inst in bb.instructions:
            if type(inst).__name__ in limited:
                s = inst.sync_info
                nw = len(s.on_wait) if s and s.on_wait else 0
                if nw > 1:
                    bad.append((inst.name, type(inst).__name__,
                                [(w.ant_name, w.wait_value)
                                 for w in s.on_wait]))
    if bad:
        raise RuntimeError(f"{len(bad)} insts with >1 wait: {bad[:8]}")


def _np_dt(name):
    if name in ("float32", "float32r"):
        return np.float32
    import ml_dtypes
    return np.dtype(getattr(ml_dtypes, name))


def make_in_maps(x, weights, bias):
    """Shard + transpose + round the full inputs into per-core in_maps."""
    dt = _np_dt(MM_DT)
    xT = np.ascontiguousarray(x.T).astype(dt)  # [IN, B]
    wT = np.ascontiguousarray(weights.T).astype(dt)  # [IN, OUT]
    in_maps = []
    for c in range(N_CORES):
        mb, nb = divmod(c, NB_SPLIT)
        in_maps.append({
            "xt": np.ascontiguousarray(xT[:, mb * BM:(mb + 1) * BM]),
            "wt": np.ascontiguousarray(wT[:, nb * NO:(nb + 1) * NO]),
            "bi": np.ascontiguousarray(bias[nb * NO:(nb + 1) * NO],
                                       dtype=np.float32),
        })
    return in_maps


def assemble_out(results):
    """Gather per-core y blocks into the full fp32 output."""
    out = np.empty((B, OUT), dtype=np.float32)
    for c in range(N_CORES):
        mb, nb = divmod(c, NB_SPLIT)
        out[mb * BM:(mb + 1) * BM,
            nb * NO:(nb + 1) * NO] = np.asarray(results[c]["y"],
                                                dtype=np.float32)
    return out


def kernel(x, weights, bias):
    from concourse.bass_utils import run_bass_kernel_spmd

    x = np.asarray(x, dtype=np.float32)
    weights = np.asarray(weights, dtype=np.float32)
    bias = np.asarray(bias, dtype=np.float32)

    if MM_DT not in _CACHE:
        _CACHE[MM_DT] = _build(MM_DT)
    nc = _CACHE[MM_DT]

    in_maps = make_in_maps(x, weights, bias)
    res = run_bass_kernel_spmd(nc, in_maps, core_ids=list(range(N_CORES)))
    return assemble_out(res.results)


# revision 12
# speedup vs baseline: 1.0140x; 1.0140x over previous
"""Trainium2 Bass kernel for y = x @ W^T + b  (B=4096, IN=OUT=2048, fp32).

Sharding: 4-way split on batch x 2-way split on out_features across the 8
NeuronCores.  Each core computes a [1024, 1024] block of the output from
x^T shard [2048, 1024] and W^T shard [2048, 1024] (both pre-transposed and
rounded to bf16 on the host; K lands on SBUF partitions with contiguous
DMAs).  bf16 inputs stream the PE at the same 1 row/cycle as fp32r but
halve HBM traffic; PSUM accumulates in fp32 so the K=2048 reduction loses
nothing, and the bf16 output rounding keeps rel-err ~4e-3.

Structure (per core, 8 PSUM banks = 4 m-tiles x 2 n-tiles per phase):
 - phase A (m-tiles 0-3): k-OUTER loop so each k-tile's w/x DMA only has
   to land just before its 8 matmuls -- compute starts after ~160KB.
 - phase B (m-tiles 4-7): k-INNER per bank (all inputs are resident by
   then), so banks complete progressively and the bias-add + store drain
   overlaps the remaining matmuls instead of serializing at the end.
 - DMA: w on the SP HWDGE queue, x/bias on the Activation HWDGE queue,
   phase-A stores on the gpsimd SWDGE queue, phase-B stores back on the
   Activation queue (idle by then).

Constraint driving the sync passes: a Matmult instruction on TRN2
supports only ONE sync-wait; extra waits are split into EventSemaphore
prefixes on the PE queue, and redundant PE self-waits are dropped.
"""

import os

import numpy as np

P = 128
B, IN, OUT = 4096, 2048, 2048
MB_SPLIT, NB_SPLIT = 4, 2  # batch-split x out-split = 8 cores
BM = B // MB_SPLIT  # 1024 batch rows per core
NO = OUT // NB_SPLIT  # 1024 out cols per core
KT = IN // P  # 16 k-tiles
MT = BM // P  # 8 m-tiles
NFREE = 512  # PSUM bank free dim (fp32)
NT = NO // NFREE  # 2 n-tiles
N_CORES = 8
HALF = (MT // 2) * P  # 512 x^T cols per phase

MM_DT = os.environ.get("BASS_MM_DT", "bfloat16")
OUT_DT = os.environ.get("BASS_OUT_DT", "bfloat16")

_CACHE = {}


def _build(mm_dt_name: str):
    import concourse.bass as bass
    import concourse.mybir as mybir
    import concourse.tile as tile

    mmdt = getattr(mybir.dt, mm_dt_name)
    outdt = getattr(mybir.dt, OUT_DT)
    f32 = mybir.dt.float32

    nc = bass.Bass("TRN2", target_bir_lowering=False, debug=False,
                   num_devices=N_CORES)
    xt = nc.dram_tensor("xt", [IN, BM], mmdt, kind="ExternalInput")
    wt = nc.dram_tensor("wt", [IN, NO], mmdt, kind="ExternalInput")
    bi = nc.dram_tensor("bi", [NO], f32, kind="ExternalInput")
    y = nc.dram_tensor("y", [BM, NO], outdt, kind="ExternalOutput")

    xt_r = xt.ap().rearrange("(k p) m -> p k m", p=P)  # [128, 16, 1024]
    wt_r = wt.ap().rearrange("(k p) n -> p k n", p=P)
    y_ap = y.ap()

    # group order within a phase: n fastest, so the DVE drain order
    # (m-major) matches phase B's bank-completion order
    groups = [(m, n) for m in range(MT // 2) for n in range(NT)]

    with tile.TileContext(nc) as tc:
        with (
            tc.tile_pool(name="xp", bufs=1) as xp,
            tc.tile_pool(name="wp", bufs=1) as wp,
            tc.tile_pool(name="bp", bufs=1) as bp,
            tc.tile_pool(name="op", bufs=1) as op,
            tc.tile_pool(name="ps", bufs=1, space="PSUM") as ps,
        ):
            # ---- input DMA emission ----
            # Full-width [128, 1024] tiles only: DMA cost is per-descriptor
            # (one per partition line), so a half-width tile costs the same
            # transfer time as a full one.  w on the SP (sync) HWDGE queue,
            # x + bias on the Activation (scalar) HWDGE queue; each x tile
            # carries both phase halves, so phase B needs no new DMAs.
            wk = [None] * KT
            xk = [None] * KT
            for k in range(KT):
                t = wp.tile([P, NO], mmdt, tag=f"wk{k}", name=f"wk{k}")
                nc.sync.dma_start(t[:], wt_r[:, k, :])
                wk[k] = t
                t = xp.tile([P, BM], mmdt, tag=f"xk{k}", name=f"xk{k}")
                nc.scalar.dma_start(t[:], xt_r[:, k, :])
                xk[k] = t
            bias_sb = bp.tile([P, NO], f32, tag="bias")
            nc.scalar.dma_start(bias_sb[:],
                                bi.ap()[None, :].to_broadcast((P, NO)))

            def lhs(k, m):  # m in 0..MT-1 (phase B uses m+4)
                return xk[k][:, m * P:(m + 1) * P]

            def rhs_w(k, n):
                return wk[k][:, n * NFREE:(n + 1) * NFREE]

            psum = {}
            for gi, g in enumerate(groups):
                psum[g] = ps.tile([P, NFREE], f32, tag=f"ps{gi}",
                                  name=f"psum_a_{gi}")

            # ---- phase A: k-outer ----
            for k in range(KT):
                for m, n in groups:
                    nc.tensor.matmul(
                        psum[(m, n)][:],
                        lhsT=lhs(k, m),
                        rhs=rhs_w(k, n),
                        start=(k == 0),
                        stop=(k == KT - 1),
                    )
            # phase A drain: one [128, NO] out tile per m; emitted in the
            # same (m, n) order as phase B consumes the banks.
            for m in range(MT // 2):
                ot = op.tile([P, NO], outdt, tag=f"outA{m}",
                             name=f"out_a_{m}")
                for n in range(NT):
                    nc.vector.tensor_add(
                        ot[:, n * NFREE:(n + 1) * NFREE],
                        psum[(m, n)][:],
                        bias_sb[:, n * NFREE:(n + 1) * NFREE])
                nc.gpsimd.dma_start(y_ap[m * P:(m + 1) * P, :], ot[:])

            # ---- phase B: k-inner per bank, progressive drain ----
            # per-(m, n) out tiles + stores so each bank's result ships as
            # soon as its add finishes; the final store is only 128KB.
            for gi, (m, n) in enumerate(groups):
                t = ps.tile([P, NFREE], f32, tag=f"ps{gi}",
                            name=f"psum_b_{gi}")
                for k in range(KT):
                    nc.tensor.matmul(
                        t[:],
                        lhsT=lhs(k, MT // 2 + m),
                        rhs=rhs_w(k, n),
                        start=(k == 0),
                        stop=(k == KT - 1),
                    )
                ot = op.tile([P, NFREE], outdt, tag=f"outB{gi}",
                             name=f"out_b_{gi}")
                nc.vector.tensor_add(
                    ot[:], t[:],
                    bias_sb[:, n * NFREE:(n + 1) * NFREE])
                row0 = (MT // 2 + m) * P
                nc.scalar.dma_start(
                    y_ap[row0:row0 + P, n * NFREE:(n + 1) * NFREE], ot[:])

    _strip_redundant_pe_waits(nc)
    _legalize_multi_waits(nc)
    _check_matmul_waits(nc)
    return nc


def _legalize_multi_waits(nc):
    """Split multi-wait instructions into single-wait EventSemaphore
    prefixes on the same engine.

    This walrus pipeline (bass pass list, no lower_sync) supports exactly
    ONE sync wait per instruction.  A chain of EventSemaphore waits on the
    issuing engine followed by the instruction with the final wait is
    semantically identical: the engine's sequencer blocks on each in
    order.
    """
    import copy

    import concourse.mybir as mybir

    m = nc.m
    new_module = copy.replace(m, functions=[])
    counter = [0]
    for function in m.functions:
        new_function = copy.replace(function, blocks=[])
        new_function.set_allocations_from_list(function.allocations)
        for block in function.blocks:
            new_insts = []
            for inst in block.instructions:
                s = inst.sync_info
                if s and s.on_wait and len(s.on_wait) > 1:
                    for w in s.on_wait[:-1]:
                        counter[0] += 1
                        ev = mybir.InstEventSemaphore(
                            name=f"legalize_wait_{counter[0]}",
                            ins=[], outs=[],
                            sync_info=mybir.SyncInfo(on_wait=[w],
                                                     on_update=[]),
                            engine=inst.engine,
                        )
                        new_insts.append(ev)
                    inst.sync_info = mybir.SyncInfo(
                        on_wait=[s.on_wait[-1]], on_update=s.on_update)
                new_insts.append(inst)
            new_function.blocks.append(
                copy.replace(block, instructions=new_insts))
        new_module.functions.append(new_function)
    nc.m = new_module


def _strip_redundant_pe_waits(nc):
    """Drop PE self-waits on matmuls that also wait on the DVE release.

    TRN2 matmuls support one sync wait.  Tile's wait emission is not
    transitively minimal: a PSUM-bank reuse emits both the bank's last PE
    writer (self-engine, redundant: the DVE add that releases the bank
    already waits on that writer) and the DVE release.  Keeping the DVE
    wait preserves the hazard ordering.
    """
    import concourse.mybir as mybir

    for bb in nc.m.functions[0].blocks:
        for inst in bb.instructions:
            if type(inst).__name__ != "InstMatmult":
                continue
            s = inst.sync_info
            if not (s and s.on_wait and len(s.on_wait) > 1):
                continue
            keep = [w for w in s.on_wait if not w.ant_name.startswith("PE")]
            dve = [w for w in keep if w.ant_name.startswith("DVE")]
            if len(keep) == len(s.on_wait) - 1 and dve:
                inst.sync_info = mybir.SyncInfo(on_wait=keep,
                                                on_update=s.on_update)


def _check_matmul_waits(nc):
    """TRN2 compute instructions (Matmult, TensorTensor, ...) support one
    sync wait; walrus codegen hard-fails on more."""
    limited = {"InstMatmult", "InstTensorTensor", "InstTensorScalarPtr",
               "InstActivation", "InstTensorCopy", "InstCopy"}
    bad = []
    for bb in nc.m.functions[0].blocks:
        for inst in bb.instructions:
            if type(inst).__name__ in limited:
                s = inst.sync_info
                nw = len(s.on_wait) if s and s.on_wait else 0
                if nw > 1:
                    bad.append((inst.name, type(inst).__name__,
                                [(w.ant_name, w.wait_value)
                                 for w in s.on_wait]))
    if bad:
        raise RuntimeError(f"{len(bad)} insts with >1 wait: {bad[:8]}")


def _np_dt(name):
    if name in ("float32", "float32r"):
        return np.float32
    import ml_dtypes
    return np.dtype(getattr(ml_dtypes, name))


def make_in_maps(x, weights, bias):
    """Shard + transpose + round the full inputs into per-core in_maps."""
    dt = _np_dt(MM_DT)
    xT = np.ascontiguousarray(x.T).astype(dt)  # [IN, B]
    wT = np.ascontiguousarray(weights.T).astype(dt)  # [IN, OUT]
    in_maps = []
    for c in range(N_CORES):
        mb, nb = divmod(c, NB_SPLIT)
        in_maps.append({
            "xt": np.ascontiguousarray(xT[:, mb * BM:(mb + 1) * BM]),
            "wt": np.ascontiguousarray(wT[:, nb * NO:(nb + 1) * NO]),
            "bi": np.ascontiguousarray(bias[nb * NO:(nb + 1) * NO],
                                       dtype=np.float32),
        })
    return in_maps


def assemble_out(results):
    """Gather per-core y blocks into the full fp32 output."""
    out = np.empty((B, OUT), dtype=np.float32)
    for c in range(N_CORES):
        mb, nb = divmod(c, NB_SPLIT)
        out[mb * BM:(mb + 1) * BM,
            nb * NO:(nb + 1) * NO] = np.asarray(results[c]["y"],
                                                dtype=np.float32)
    return out


def kernel(x, weights, bias):
    from concourse.bass_utils import run_bass_kernel_spmd

    x = np.asarray(x, dtype=np.float32)
    weights = np.asarray(weights, dtype=np.float32)
    bias = np.asarray(bias, dtype=np.float32)

    if MM_DT not in _CACHE:
        _CACHE[MM_DT] = _build(MM_DT)
    nc = _CACHE[MM_DT]

    in_maps = make_in_maps(x, weights, bias)
    res = run_bass_kernel_spmd(nc, in_maps, core_ids=list(range(N_CORES)))
    return assemble_out(res.results)
